# revision 90
# baseline (speedup 1.0000x reference)
"""Additive (Bahdanau) attention kernel for 8 TRN2 NeuronCores.

Problem (full shapes): H=1024, B=64, S=2048
    enc = transpose(encoder_states, (1,0,2))            # (B,S,H)
    proj_prev = decoder_prev_state @ Wp.T               # (B,H)
    proj_enc  = enc @ We.T                              # (B,S,H)
    scores    = einsum('bsh,h->bs', tanh(pp+pe), v)     # (B,S)
    attn      = softmax(where(mask==0, -inf, scores))
    out       = einsum('bsh,bs->bh', enc, attn)         # (B,H)

Sharding: data-parallel over batch. Each of the 8 cores handles 8 batch
rows; the small weight matrices are replicated. No collectives.

Host-side preprocessing (layout only, no arithmetic):
  - weights are passed pre-transposed (WeT = We.T, WpT = Wp.T contiguous)
    so the kernel loads W^T rows straight into the h-major layout the PE
    needs, instead of transposing 128x128 blocks on the PE at startup;
  - masked-out s positions contribute exactly nothing (exp(score)*0) to
    the softmax and the context sum, so each batch row's unmasked encoder
    rows are compacted (host gather) and padded to a 128-multiple bucket
    s_pad = max_b count(mask_b) rounded up. The device kernel runs on
    s_pad <= S columns with the compacted 0/1 mask applied as before, so
    correctness never depends on the compaction (dense masks simply use
    the full 2048 bucket). Work scales by s_pad/S (~0.56 for ~half-dense
    random masks).

Since enc arrives host-transposed ([b, h, s']), tiles load h-major with
2KB-contiguous DMA lines and no on-chip transpose is needed anywhere.
Ragged tails of g_n batch rows are packed into one full 512-column tile
(tail batching) so every projection matmul runs at N=512.

Per-core dataflow (all matmuls bf16 on the PE, f32 PSUM accumulate):
  - SWDGE cast-load of an h-major enc tile          -> bf16 [h, s'] (512 s')
  - proj^T[m,:] = sum_k WeT[k,m].T @ enc^T[k]  (8xk matmuls, N=512)
  - ScalarE: tanh(psum + qprojT[:,b]) fused via activation bias (per-b
    column segment in tail-batched tiles)
  - score   = sum_m vT[m].T @ tanh[m]         (matvec matmuls, M=1)
  - p = exp(score) * maskf  (no max subtraction needed: |score| <= ||v||*32,
    exp stays finite in f32); denominator via reduce_sum per b segment
  - p broadcast to all partitions by a K=1 ones-matmul; context numerator
    accumulates on the otherwise-idle DVE as mult+reduce over the s axis
    (keeps the matvec matmuls off the PE, the bottleneck engine)
  - finalize per b, split so no PE op ever waits on the divide chain:
    num/den with den broadcast via ones-matmul, one PE transpose to land
    [h,k] as [k,h] rows, one staged store at kernel end
"""

import numpy as np

H = 1024
B = 64
S = 2048
NCORES = 8
BL = B // NCORES  # 8 batch rows per core
P = 128
ST = 512          # max s-tile
KC = H // P       # 8 h-chunks
C4 = ST // P      # partition-chunks in a full s-tile

_CACHE = {}
LOAD_MODE = "swdge"  # "swdge": cast-load in DMA; "hwdge": f32 load + Pool cast


def _tile_sizes(s_pad):
    """Split s_pad into s-tiles: full 512s plus one 128-multiple tail."""
    sizes = [ST] * (s_pad // ST)
    if s_pad % ST:
        sizes.append(s_pad % ST)
    return sizes


def _build_bass(reps=1, s_pad=S, variant="full"):
    """Build the Bass module for a given padded s length.

    reps>1 unrolls the complete per-core kernel (including weight staging)
    that many times inside one NEFF, so a single device launch executes the
    whole problem `reps` times back-to-back — used by the benchmark to
    measure steady-state per-execution device time with host dispatch
    amortized away."""
    import concourse.bass as bass
    import concourse.mybir as mybir
    import concourse.tile as tile

    fp32 = mybir.dt.float32
    bf16 = mybir.dt.bfloat16
    i32 = mybir.dt.int32
    Tanh = mybir.ActivationFunctionType.Tanh
    Exp = mybir.ActivationFunctionType.Exp
    mult = mybir.AluOpType.mult

    sizes = _tile_sizes(s_pad)
    ntiles = len(sizes)
    offs = [sum(sizes[:i]) for i in range(ntiles)]
    # tail batching (see main loop): pack g_n batch-rows' tails into one
    # full 512-column tile when the tail divides 512
    tail = s_pad % ST
    nfull = s_pad // ST
    g_n = ST // tail if (tail and ST % tail == 0 and nfull) else 1
    tc = tail // P if g_n > 1 else 0
    nper = nfull + (1 if (g_n == 1 and tail) else 0)  # per-b tile count

    nc = bass.Bass()

    # enc is passed host-transposed: [b, h, s'] so h-major tiles load with
    # 2KB-contiguous DMA lines (no on-chip transpose needed at all)
    enc = nc.dram_tensor("encoder_states", [BL, H, s_pad], fp32,
                         kind="ExternalInput")
    dec = nc.dram_tensor("decoder_prev_state", [BL, H], fp32,
                         kind="ExternalInput")
    msk = nc.dram_tensor("mask", [BL, s_pad], i32, kind="ExternalInput")
    WpT_d = nc.dram_tensor("WpT", [H, H], fp32, kind="ExternalInput")
    WeT_d = nc.dram_tensor("WeT", [H, H], fp32, kind="ExternalInput")
    v = nc.dram_tensor("v", [H], fp32, kind="ExternalInput")
    out = nc.dram_tensor("out", [BL, H], fp32, kind="ExternalOutput")

    with tile.TileContext(nc) as tc:
        with (
            tc.tile_pool(name="consts", bufs=1) as consts,
            tc.tile_pool(name="xf", bufs=2) as xf_pool,
            tc.tile_pool(name="xt", bufs=4) as xt_pool,
            tc.tile_pool(name="th", bufs=4) as th_pool,
            tc.tile_pool(name="sm", bufs=4) as sm,
            tc.tile_pool(name="pp", bufs=3) as pp_pool,
            tc.tile_pool(name="pj", bufs=4, space="PSUM") as psum_pj,
            tc.tile_pool(name="ps", bufs=2, space="PSUM") as psum_s,
            tc.tile_pool(name="pn", bufs=2, space="PSUM") as psum_n,
        ):
            def load_tile(b, st):
                size = sizes[st]
                # h-major load: xh[p, k, s'] = enc[b, k*128+p, off+s']
                # (per-partition lines are 4B*size contiguous)
                src = enc[b, :, offs[st]:offs[st] + size].rearrange(
                    "(k p) s -> p k s", p=P
                )
                xh = xt_pool.tile([P, KC, ST], bf16, tag="xt", name="xt")
                if LOAD_MODE == "swdge":
                    nc.gpsimd.dma_start(out=xh[:, :, 0:size], in_=src)
                else:
                    xf = xf_pool.tile([P, KC, ST], fp32, tag="xf", name="xf")
                    nc.sync.dma_start(out=xf[:, :, 0:size], in_=src)
                    nc.gpsimd.tensor_copy(out=xh[:, :, 0:size],
                                          in_=xf[:, :, 0:size])
                return xh

            for _rep in range(reps):
                # ---------- setup ----------
                from concourse.masks import make_identity
                ident = consts.tile([P, P], bf16, tag="ident", name="ident")
                make_identity(nc, ident[:])

                # First tile's load goes ahead of the weight loads so the
                # PE can start as soon as WeT lands.
                pre = {(0, 0): load_tile(0, 0)}

                # WT[p, k, m] = W^T[k*128+p, m]  (bf16 cast-load, row DMAs)
                WeT = consts.tile([P, KC, H], bf16, tag="WeT", name="WeT")
                WpT = consts.tile([P, KC, H], bf16, tag="WpT", name="WpT")
                for k in range(KC):
                    nc.gpsimd.dma_start(
                        out=WeT[:, k, :], in_=WeT_d[k * P:(k + 1) * P, :]
                    )
                dec_bf = consts.tile([16, H], bf16, tag="dec_bf", name="dec_bf")
                nc.vector.memset(dec_bf[:], 0.0)
                nc.gpsimd.dma_start(out=dec_bf[0:BL, :], in_=dec[:, :])
                v_sb = consts.tile([16, H], bf16, tag="v_sb", name="v_sb")
                nc.vector.memset(v_sb[:], 0.0)
                nc.gpsimd.dma_start(out=v_sb[0:1, :], in_=v[:])
                for k in range(KC):
                    nc.gpsimd.dma_start(
                        out=WpT[:, k, :], in_=WpT_d[k * P:(k + 1) * P, :]
                    )
                decTt = consts.tile([P, KC, 16], bf16, tag="decTt", name="decTt")
                vT = consts.tile([P, KC, 16], bf16, tag="vT", name="vT")

                if nper > 1:
                    pre[(0, 1)] = load_tile(0, 1)

                # decTt[p, k, b] = dec[b, k*128+p]; vT[p, k, 0] = v[k*128+p]
                for k in range(KC):
                    tps = psum_pj.tile([P, 512], bf16, tag="pj", name="tps")
                    nc.tensor.transpose(
                        tps[:, 0:16], dec_bf[:, k * P:(k + 1) * P],
                        ident[0:16, 0:16]
                    )
                    nc.vector.tensor_copy(out=decTt[:, k, :], in_=tps[:, 0:16])
                    tps2 = psum_pj.tile([P, 512], bf16, tag="pj", name="tps2")
                    nc.tensor.transpose(
                        tps2[:, 0:16], v_sb[:, k * P:(k + 1) * P],
                        ident[0:16, 0:16]
                    )
                    nc.vector.tensor_copy(out=vT[:, k, :], in_=tps2[:, 0:16])
                if nper > 2:
                    pre[(0, 2)] = load_tile(0, 2)

                # qprojT[p, mc, b] = (dec @ Wp.T)[b, mc*128+p]
                qprojT = consts.tile([P, KC, BL], fp32, tag="qprojT",
                                     name="qprojT")
                for mc in range(KC):
                    pq = psum_pj.tile([P, 512], fp32, tag="pj", name="pq")
                    for k in range(KC):
                        nc.tensor.matmul(
                            pq[:, 0:BL],
                            lhsT=WpT[:, k, mc * P:(mc + 1) * P],
                            rhs=decTt[:, k, 0:BL],
                            start=(k == 0),
                            stop=(k == KC - 1),
                        )
                    nc.vector.tensor_copy(out=qprojT[:, mc, :], in_=pq[:, 0:BL])

                # all-ones row for partition-broadcast matmuls (K=1)
                ones1 = consts.tile([1, P], bf16, tag="ones1", name="ones1")
                nc.vector.memset(ones1[:], 1.0)
                # f32 identity for the final [p,k]->[k,p] PE transpose
                identf = consts.tile([P, P], fp32, tag="identf", name="identf")
                make_identity(nc, identf[:])
                # final output staging: outstage[k, b*128+f] = out[b, k*128+f]
                outstage = consts.tile([KC, BL * P], fp32, tag="outstage",
                                       name="outstage")
                if variant != "full":
                    nc.vector.memset(outstage[:], 0.0)

                # ---------- main loop ----------
                def finalize_a(den):
                    # den total -> bf16 scalar (DVE only; emitted at b end)
                    dtot = sm.tile([1, 1], fp32, tag="dtot", name="dtot")
                    nc.vector.reduce_sum(out=dtot[:], in_=den[:],
                                         axis=mybir.AxisListType.X)
                    dbf = sm.tile([1, 1], bf16, tag="dbf", name="dbf", bufs=2)
                    nc.vector.tensor_copy(out=dbf[:], in_=dtot[:])
                    return dbf

                def finalize_b(b, acc, dbf):
                    # out[b] = num / den.  The PE transpose depends only on
                    # acc (done at b end) and the den broadcast only on dbf,
                    # so neither stalls the PE stream when emitted later.
                    dps = psum_n.tile([P, ST], fp32, tag="pbc", name="dps")
                    nc.tensor.matmul(
                        dps[:, 0:1], lhsT=ones1[:], rhs=dbf[:],
                        start=True, stop=True
                    )
                    cps = psum_s.tile([KC, 512], fp32, tag="ps", name="cps")
                    nc.tensor.transpose(cps[:, 0:P], acc[:], identf[:])
                    inv = sm.tile([KC, 1], fp32, tag="inv", name="inv")
                    nc.vector.reciprocal(out=inv[:], in_=dps[0:KC, 0:1])
                    nc.vector.tensor_scalar_mul(
                        outstage[:, b * P:(b + 1) * P], cps[0:KC, 0:P], inv[:]
                    )

                # ---- main loop with tail batching ----
                # If the tail (s_pad % 512) divides 512, the tails of
                # g_n = 512//tail consecutive batch rows are packed into ONE
                # full 512-column tile: the projection/score matmuls are
                # batch-agnostic (same We, same v), only the tanh bias, the
                # mask and the softmax segmentation are per-b — applied on
                # column segments. This removes all narrow (N<512) matmuls.
                def proj_tile(xh, size, bias_segs, ps, matmul_only=False):
                    # proj^T + fused tanh(x + qproj_b) per column segment.
                    # The score matmul for group mc is emitted AFTER group
                    # mc+1's projection matmuls: the in-order PE queue would
                    # otherwise stall on the tanh(mc) -> score(mc)
                    # dependency eight times per tile.
                    def score(mc, th):
                        # scores[0, s'] += sum_h v[h] * tanh[h, s']
                        nc.tensor.matmul(
                            ps[:, 0:size],
                            lhsT=vT[:, mc, 0:1],
                            rhs=th[:, 0:size],
                            start=(mc == 0),
                            stop=(mc == KC - 1),
                        )

                    prev = None
                    for mc in range(KC):
                        pj = psum_pj.tile([P, 512], fp32, tag="pj", name="pj")
                        for k in range(KC):
                            nc.tensor.matmul(
                                pj[:, 0:size],
                                lhsT=WeT[:, k, mc * P:(mc + 1) * P],
                                rhs=xh[:, k, 0:size],
                                start=(k == 0),
                                stop=(k == KC - 1),
                            )
                        if matmul_only:
                            continue  # timing-only: raw matmul stream
                        if prev is not None:
                            score(*prev)
                        th = th_pool.tile([P, ST], bf16, tag="th", name="th")
                        for (b, so, sz) in bias_segs:
                            nc.scalar.activation(
                                out=th[:, so:so + sz],
                                in_=pj[:, so:so + sz],
                                func=Tanh,
                                bias=qprojT[:, mc, b:b + 1],
                                scale=1.0,
                            )
                        prev = (mc, th)
                    if not matmul_only:
                        score(*prev)

                def numer_tile(xh, size, pv, segs):
                    """Shared broadcast + products; per-(b,segment)
                    reduction into each b's acc. segs: (acc, s0, slen)."""
                    pbc = psum_n.tile([P, ST], fp32, tag="pbc", name="pbc")
                    nc.tensor.matmul(
                        pbc[:, 0:size], lhsT=ones1[:], rhs=pv[:, 0:size],
                        start=True, stop=True
                    )
                    tmp = pp_pool.tile([P, KC, ST], bf16, tag="ntmp",
                                       name="ntmp", bufs=2)
                    nc.vector.tensor_tensor(
                        out=tmp[:, :, 0:size],
                        in0=xh[:, :, 0:size],
                        in1=pbc[:, 0:size][:, None, :].to_broadcast(
                            [P, KC, size]),
                        op=mult,
                    )
                    for (acc, s0, slen) in segs:
                        red = sm.tile([P, KC], fp32, tag="red", name="red",
                                      bufs=2)
                        nc.vector.reduce_sum(
                            out=red[:], in_=tmp[:, :, s0:s0 + slen],
                            axis=mybir.AxisListType.X
                        )
                        nc.vector.tensor_add(out=acc[:], in0=acc[:],
                                             in1=red[:])

                ngroups = BL // g_n
                nden = nfull + (1 if tail else 0)
                pending = []
                for g in range(ngroups):
                    bs = list(range(g * g_n, (g + 1) * g_n))
                    accs, dens, mrows = {}, {}, {}
                    for b in bs:
                        accs[b] = sm.tile([P, KC], fp32, tag="acc",
                                          name="acc", bufs=2 * g_n)
                        nc.vector.memset(accs[b][:], 0.0)
                        dens[b] = sm.tile([1, max(nden, 1)], fp32, tag="den",
                                          name="den", bufs=2 * g_n)
                        # mask row via SWDGE: the sync/HWDGE queue stays in
                        # xbar transpose mode all rep long (copy<->transpose
                        # switches drain the xbar ~3us, not in the model)
                        mrows[b] = sm.tile([1, S], i32, tag="mrow",
                                           name="mrow", bufs=g_n + 1)
                        nc.gpsimd.dma_start(out=mrows[b][:, 0:s_pad],
                                            in_=msk[b:b + 1, :])
                    for b in bs:
                        for st in range(nper):
                            size = sizes[st]
                            if (b, st) in pre:
                                xh = pre.pop((b, st))
                            else:
                                xh = load_tile(b, st)

                            if pending:
                                finalize_b(*pending.pop(0))
                            if variant == "loads":
                                continue
                            ps = psum_s.tile([1, 512], fp32, tag="ps",
                                             name="ps")
                            proj_tile(xh, size, [(b, 0, size)], ps,
                                      matmul_only=(variant == "projonly"))
                            if variant in ("pescore", "projonly"):
                                continue

                            ex = sm.tile([1, ST], fp32, tag="ex", name="ex",
                                         bufs=2)
                            nc.scalar.activation(out=ex[:, 0:size],
                                                 in_=ps[:, 0:size], func=Exp)
                            mf = sm.tile([1, ST], fp32, tag="mf", name="mf",
                                         bufs=2)
                            nc.vector.tensor_copy(
                                out=mf[:, 0:size],
                                in_=mrows[b][0:1, offs[st]:offs[st] + size]
                            )
                            # p = ex * maskf (bf16); den[st] = sum_s p
                            pv = pp_pool.tile([1, ST], bf16, tag="pv",
                                              name="pv")
                            nc.vector.tensor_tensor(
                                out=pv[:, 0:size], in0=ex[:, 0:size],
                                in1=mf[:, 0:size], op=mult
                            )
                            nc.vector.reduce_sum(
                                out=dens[b][:, st:st + 1],
                                in_=pv[:, 0:size],
                                axis=mybir.AxisListType.X,
                            )
                            numer_tile(xh, size, pv, [(accs[b], 0, size)])

                    if g_n > 1:
                        # shared tail tile: g_n tails side by side, full 512
                        xh = xt_pool.tile([P, KC, ST], bf16, tag="xt",
                                          name="xt")
                        if LOAD_MODE == "swdge":
                            for j, b in enumerate(bs):
                                nc.gpsimd.dma_start(
                                    out=xh[:, :, j * tail:(j + 1) * tail],
                                    in_=enc[b, :,
                                            offs[nfull]:offs[nfull] + tail
                                            ].rearrange("(k p) t -> p k t",
                                                        p=P),
                                )
                        else:
                            xf = xf_pool.tile([P, KC, ST], fp32, tag="xf",
                                              name="xf")
                            for j, b in enumerate(bs):
                                nc.sync.dma_start(
                                    out=xf[:, :, j * tail:(j + 1) * tail],
                                    in_=enc[b, :,
                                            offs[nfull]:offs[nfull] + tail
                                            ].rearrange("(k p) t -> p k t",
                                                        p=P),
                                )
                            nc.gpsimd.tensor_copy(out=xh[:], in_=xf[:])
                        if pending:
                            finalize_b(*pending.pop(0))
                        if variant != "loads":
                            ps = psum_s.tile([1, 512], fp32, tag="ps",
                                             name="ps")
                            proj_tile(xh, ST,
                                      [(b, j * tail, tail)
                                       for j, b in enumerate(bs)], ps,
                                      matmul_only=(variant == "projonly"))
                        if variant == "full":
                            ex = sm.tile([1, ST], fp32, tag="ex", name="ex",
                                         bufs=2)
                            nc.scalar.activation(out=ex[:], in_=ps[:],
                                                 func=Exp)
                            mf = sm.tile([1, ST], fp32, tag="mf", name="mf",
                                         bufs=2)
                            for j, b in enumerate(bs):
                                nc.vector.tensor_copy(
                                    out=mf[:, j * tail:(j + 1) * tail],
                                    in_=mrows[b][0:1,
                                                 offs[nfull]:offs[nfull]
                                                 + tail]
                                )
                            pv = pp_pool.tile([1, ST], bf16, tag="pv",
                                              name="pv")
                            nc.vector.tensor_tensor(
                                out=pv[:], in0=ex[:], in1=mf[:], op=mult
                            )
                            for j, b in enumerate(bs):
                                nc.vector.reduce_sum(
                                    out=dens[b][:, nfull:nfull + 1],
                                    in_=pv[:, j * tail:(j + 1) * tail],
                                    axis=mybir.AxisListType.X,
                                )
                            numer_tile(xh, ST, pv,
                                       [(accs[b], j * tail, tail)
                                        for j, b in enumerate(bs)])

                    if variant == "full":
                        for b in bs:
                            pending.append((b, accs[b], finalize_a(dens[b])))
                while pending:
                    finalize_b(*pending.pop(0))

                # final store via SWDGE for the same xbar-mode reason
                nc.gpsimd.dma_start(
                    out=out[:, :].rearrange("b (k f) -> k b f", k=KC),
                    in_=outstage[:].rearrange("k (b f) -> k b f", b=BL),
                )

    _legalize_dma_waits(nc)
    return nc


def _legalize_dma_waits(nc):
    """This container's walrus enforces per-instruction sync budgets the Tile
    pipeline does not respect: most ISA encodings carry at most ONE sync-wait
    slot (EventSemaphore holds two), and the 64-byte-padded
    EVENT_SEMAPHORE_RANGE_CLEAR InstISA is rejected outright.  Legalize after
    Tile: move excess waits onto standalone EventSemaphore instructions
    inserted just before the instruction on the same engine stream (the
    sequencer executes them in order, so the instruction still issues only
    after all its waits are satisfied), and drop the teardown range-clear
    (this NEFF executes once; semaphores are not recycled afterwards)."""
    import concourse.mybir as mybir
    import bass_rust

    nev = [0]

    def mkev(engine, waits, updates=()):
        ev = mybir.InstEventSemaphore(name=f"evw-{nev[0]}", ins=[], outs=[])
        nev[0] += 1
        ev.engine = engine
        ev.sync_info = bass_rust.SyncInfo(
            on_wait=list(waits), on_update=list(updates)
        )
        return ev

    for blk in nc.m.functions[0].blocks:
        insts = blk.instructions
        new = []
        for inst in insts:
            t = type(inst).__name__
            si = getattr(inst, "sync_info", None)
            cap = 2 if t == "InstEventSemaphore" else 1
            if si is not None and len(si.on_wait) > cap:
                waits = list(si.on_wait)
                extra, keep = waits[:-cap], waits[-cap:]
                for j in range(0, len(extra), 2):
                    new.append(mkev(inst.engine, extra[j:j + 2]))
                inst.sync_info = bass_rust.SyncInfo(
                    on_wait=keep, on_update=list(si.on_update)
                )
            if t == "InstISA" and getattr(inst, "op_name", "") == (
                "EVENT_SEMAPHORE_RANGE_CLEAR"
            ):
                # Replace with per-semaphore EventSemaphore writes of 0: the
                # tail barrier recycles these sem ids and expects them
                # cleared; dropping the clear leaves DMA-lane counts behind
                # and lets the final barrier pass early (intermittent
                # exec-unit errors with the output store still in flight).
                ib = list(inst.instr)
                lo, hi = ib[13], ib[14]
                for s in range(lo, hi + 1):
                    new.append(mkev(inst.engine, [], [bass_rust.SyncUpdate(
                        sync_type="semaphore", id=s, ant_name=f"semclr{s}",
                        update_mode="sem-wr-imm", update_value=0,
                        update_reg=None)]))
                continue
            new.append(inst)
        try:
            blk.instructions = new
        except Exception:
            insts.clear()
            insts.extend(new)


def _get_nc(s_pad=S, reps=1):
    key = (s_pad, reps)
    if key not in _CACHE:
        _CACHE[key] = _build_bass(reps=reps, s_pad=s_pad)
    return _CACHE[key]


def _make_in_maps(inputs):
    """Shard over batch; host-side layout prep (transpose weights, compact
    the masked s axis — pure gather/permutation, no arithmetic).
    Returns (in_maps, s_pad)."""
    enc = np.ascontiguousarray(np.asarray(inputs["encoder_states"],
                                          dtype=np.float32))
    dec = np.ascontiguousarray(np.asarray(inputs["decoder_prev_state"],
                                          dtype=np.float32))
    msk = np.ascontiguousarray(np.asarray(inputs["mask"], dtype=np.int32))
    WpT = np.ascontiguousarray(np.asarray(inputs["Wp"], dtype=np.float32).T)
    WeT = np.ascontiguousarray(np.asarray(inputs["We"], dtype=np.float32).T)
    v = np.ascontiguousarray(np.asarray(inputs["v"], dtype=np.float32))

    cnts = (msk != 0).sum(axis=1)
    s_pad = int(min(S, max(P, -(-int(cnts.max()) // P) * P)))
    # enc_t[b, h, s'] = enc[idx_b[s'], b, h]: compacted along s AND
    # transposed to h-major so device tiles load with contiguous lines
    enc_t = np.zeros((B, H, s_pad), dtype=np.float32)
    msk_c = np.zeros((B, s_pad), dtype=np.int32)
    for b in range(B):
        idx = np.nonzero(msk[b])[0] if s_pad < S else np.arange(S)
        n = len(idx)
        enc_t[b, :, :n] = enc[idx, b, :].T
        msk_c[b, :n] = msk[b, idx]
    in_maps = []
    for i in range(NCORES):
        sl = slice(i * BL, (i + 1) * BL)
        in_maps.append(
            {
                "encoder_states": np.ascontiguousarray(enc_t[sl, :, :]),
                "decoder_prev_state": np.ascontiguousarray(dec[sl, :]),
                "mask": np.ascontiguousarray(msk_c[sl, :]),
                "WpT": WpT,
                "WeT": WeT,
                "v": v,
            }
        )
    return in_maps, s_pad


def kernel_profiled(trace=False, **inputs):
    """Run on 8 cores; returns (full_output, BassKernelResults)."""
    from concourse.bass_utils import run_bass_kernel_spmd

    in_maps, s_pad = _make_in_maps(inputs)
    nc = _get_nc(s_pad)
    res = run_bass_kernel_spmd(nc, in_maps, core_ids=list(range(NCORES)),
                               trace=trace)
    out = np.concatenate([r["out"] for r in res.results], axis=0)
    return out.astype(np.float32), res


def kernel(**inputs):
    out, _ = kernel_profiled(trace=False, **inputs)
    return out


# revision 93
# speedup vs baseline: 1.0476x; 1.0476x over previous
"""Additive (Bahdanau) attention kernel for 8 TRN2 NeuronCores.

Problem (full shapes): H=1024, B=64, S=2048
    enc = transpose(encoder_states, (1,0,2))            # (B,S,H)
    proj_prev = decoder_prev_state @ Wp.T               # (B,H)
    proj_enc  = enc @ We.T                              # (B,S,H)
    scores    = einsum('bsh,h->bs', tanh(pp+pe), v)     # (B,S)
    attn      = softmax(where(mask==0, -inf, scores))
    out       = einsum('bsh,bs->bh', enc, attn)         # (B,H)

Sharding: data-parallel over batch. Each of the 8 cores handles 8 batch
rows; the small weight matrices are replicated. No collectives.

Host-side preprocessing (layout only, no arithmetic):
  - weights are passed pre-transposed (WeT = We.T, WpT = Wp.T contiguous)
    so the kernel loads W^T rows straight into the h-major layout the PE
    needs, instead of transposing 128x128 blocks on the PE at startup;
  - masked-out s positions contribute exactly nothing (exp(score)*0) to
    the softmax and the context sum, so each batch row's unmasked encoder
    rows are compacted (host gather) and padded to a 128-multiple bucket
    s_pad = max_b count(mask_b) rounded up. The device kernel runs on
    s_pad <= S columns with the compacted 0/1 mask applied as before, so
    correctness never depends on the compaction (dense masks simply use
    the full 2048 bucket). Work scales by s_pad/S (~0.56 for ~half-dense
    random masks).

Since enc arrives host-transposed ([b, h, s']), tiles load h-major with
2KB-contiguous DMA lines and no on-chip transpose is needed anywhere.
Ragged tails of g_n batch rows are packed into one full 512-column tile
(tail batching) so every projection matmul runs at N=512.

Per-core dataflow (all matmuls bf16 on the PE, f32 PSUM accumulate):
  - SWDGE cast-load of an h-major enc tile          -> bf16 [h, s'] (512 s')
  - proj^T[m,:] = sum_k WeT[k,m].T @ enc^T[k]  (8xk matmuls, N=512)
  - ScalarE: tanh(psum + qprojT[:,b]) fused via activation bias (per-b
    column segment in tail-batched tiles)
  - score   = sum_m vT[m].T @ tanh[m]         (matvec matmuls, M=1)
  - p = exp(score) * maskf  (no max subtraction needed: |score| <= ||v||*32,
    exp stays finite in f32); denominator via reduce_sum per b segment
  - p broadcast to all partitions by a K=1 ones-matmul; context numerator
    accumulates on the otherwise-idle DVE as mult+reduce over the s axis
    (keeps the matvec matmuls off the PE, the bottleneck engine)
  - finalize per b, split so no PE op ever waits on the divide chain:
    num/den with den broadcast via ones-matmul, one PE transpose to land
    [h,k] as [k,h] rows, one staged store at kernel end
"""

import numpy as np

H = 1024
B = 64
S = 2048
NCORES = 8
BL = B // NCORES  # 8 batch rows per core
P = 128
ST = 512          # max s-tile
KC = H // P       # 8 h-chunks
C4 = ST // P      # partition-chunks in a full s-tile

_CACHE = {}
LOAD_MODE = "swdge"  # "swdge": cast-load in DMA; "hwdge": f32 load + Pool cast


def _tile_sizes(s_pad):
    """Split s_pad into s-tiles: full 512s plus one 128-multiple tail."""
    sizes = [ST] * (s_pad // ST)
    if s_pad % ST:
        sizes.append(s_pad % ST)
    return sizes


def _build_bass(reps=1, s_pad=S, variant="full"):
    """Build the Bass module for a given padded s length.

    reps>1 unrolls the complete per-core kernel (including weight staging)
    that many times inside one NEFF, so a single device launch executes the
    whole problem `reps` times back-to-back — used by the benchmark to
    measure steady-state per-execution device time with host dispatch
    amortized away."""
    import concourse.bass as bass
    import concourse.mybir as mybir
    import concourse.tile as tile

    fp32 = mybir.dt.float32
    bf16 = mybir.dt.bfloat16
    i32 = mybir.dt.int32
    Tanh = mybir.ActivationFunctionType.Tanh
    Exp = mybir.ActivationFunctionType.Exp
    mult = mybir.AluOpType.mult

    sizes = _tile_sizes(s_pad)
    ntiles = len(sizes)
    offs = [sum(sizes[:i]) for i in range(ntiles)]
    # tail batching (see main loop): pack g_n batch-rows' tails into one
    # full 512-column tile when the tail divides 512
    tail = s_pad % ST
    nfull = s_pad // ST
    # pack as many batch rows' tails as fit in one <=512-column tile
    # (s' is a free dim since the h-major layout: no alignment needed)
    g_n = 1
    if tail and nfull:
        cap = ST // tail
        for d in (8, 4, 2, 1):
            if d <= cap and BL % d == 0:
                g_n = d
                break
    gsz = g_n * tail  # tail-group tile width
    nper = nfull + (1 if (g_n == 1 and tail) else 0)  # per-b tile count

    nc = bass.Bass()

    # enc is passed host-transposed: [b, h, s'] so h-major tiles load with
    # 2KB-contiguous DMA lines (no on-chip transpose needed at all)
    enc = nc.dram_tensor("encoder_states", [BL, H, s_pad], fp32,
                         kind="ExternalInput")
    dec = nc.dram_tensor("decoder_prev_state", [BL, H], fp32,
                         kind="ExternalInput")
    msk = nc.dram_tensor("mask", [BL, s_pad], i32, kind="ExternalInput")
    WpT_d = nc.dram_tensor("WpT", [H, H], fp32, kind="ExternalInput")
    WeT_d = nc.dram_tensor("WeT", [H, H], fp32, kind="ExternalInput")
    v = nc.dram_tensor("v", [H], fp32, kind="ExternalInput")
    out = nc.dram_tensor("out", [BL, H], fp32, kind="ExternalOutput")

    with tile.TileContext(nc) as tc:
        with (
            tc.tile_pool(name="consts", bufs=1) as consts,
            tc.tile_pool(name="xf", bufs=2) as xf_pool,
            tc.tile_pool(name="xt", bufs=4) as xt_pool,
            tc.tile_pool(name="th", bufs=4) as th_pool,
            tc.tile_pool(name="sm", bufs=4) as sm,
            tc.tile_pool(name="pp", bufs=3) as pp_pool,
            tc.tile_pool(name="pj", bufs=4, space="PSUM") as psum_pj,
            tc.tile_pool(name="ps", bufs=2, space="PSUM") as psum_s,
            tc.tile_pool(name="pn", bufs=2, space="PSUM") as psum_n,
        ):
            def load_tile(b, st):
                size = sizes[st]
                # h-major load: xh[p, k, s'] = enc[b, k*128+p, off+s']
                # (per-partition lines are 4B*size contiguous)
                src = enc[b, :, offs[st]:offs[st] + size].rearrange(
                    "(k p) s -> p k s", p=P
                )
                xh = xt_pool.tile([P, KC, ST], bf16, tag="xt", name="xt")
                if LOAD_MODE == "swdge":
                    nc.gpsimd.dma_start(out=xh[:, :, 0:size], in_=src)
                else:
                    xf = xf_pool.tile([P, KC, ST], fp32, tag="xf", name="xf")
                    nc.sync.dma_start(out=xf[:, :, 0:size], in_=src)
                    nc.gpsimd.tensor_copy(out=xh[:, :, 0:size],
                                          in_=xf[:, :, 0:size])
                return xh

            for _rep in range(reps):
                # ---------- setup ----------
                from concourse.masks import make_identity
                ident = consts.tile([P, P], bf16, tag="ident", name="ident")
                make_identity(nc, ident[:])

                # First tile's load goes ahead of the weight loads so the
                # PE can start as soon as WeT lands.
                pre = {(0, 0): load_tile(0, 0)}

                # WT[p, k, m] = W^T[k*128+p, m]  (bf16 cast-load, row DMAs)
                WeT = consts.tile([P, KC, H], bf16, tag="WeT", name="WeT")
                WpT = consts.tile([P, KC, H], bf16, tag="WpT", name="WpT")
                for k in range(KC):
                    nc.gpsimd.dma_start(
                        out=WeT[:, k, :], in_=WeT_d[k * P:(k + 1) * P, :]
                    )
                dec_bf = consts.tile([16, H], bf16, tag="dec_bf", name="dec_bf")
                nc.vector.memset(dec_bf[:], 0.0)
                nc.gpsimd.dma_start(out=dec_bf[0:BL, :], in_=dec[:, :])
                v_sb = consts.tile([16, H], bf16, tag="v_sb", name="v_sb")
                nc.vector.memset(v_sb[:], 0.0)
                nc.gpsimd.dma_start(out=v_sb[0:1, :], in_=v[:])
                for k in range(KC):
                    nc.gpsimd.dma_start(
                        out=WpT[:, k, :], in_=WpT_d[k * P:(k + 1) * P, :]
                    )
                decTt = consts.tile([P, KC, 16], bf16, tag="decTt", name="decTt")
                vT = consts.tile([P, KC, 16], bf16, tag="vT", name="vT")

                if nper > 1:
                    pre[(0, 1)] = load_tile(0, 1)

                # decTt[p, k, b] = dec[b, k*128+p]; vT[p, k, 0] = v[k*128+p]
                for k in range(KC):
                    tps = psum_pj.tile([P, 512], bf16, tag="pj", name="tps")
                    nc.tensor.transpose(
                        tps[:, 0:16], dec_bf[:, k * P:(k + 1) * P],
                        ident[0:16, 0:16]
                    )
                    nc.vector.tensor_copy(out=decTt[:, k, :], in_=tps[:, 0:16])
                    tps2 = psum_pj.tile([P, 512], bf16, tag="pj", name="tps2")
                    nc.tensor.transpose(
                        tps2[:, 0:16], v_sb[:, k * P:(k + 1) * P],
                        ident[0:16, 0:16]
                    )
                    nc.vector.tensor_copy(out=vT[:, k, :], in_=tps2[:, 0:16])
                if nper > 2:
                    pre[(0, 2)] = load_tile(0, 2)

                # qprojT[p, mc, b] = (dec @ Wp.T)[b, mc*128+p]
                qprojT = consts.tile([P, KC, BL], fp32, tag="qprojT",
                                     name="qprojT")
                for mc in range(KC):
                    pq = psum_pj.tile([P, 512], fp32, tag="pj", name="pq")
                    for k in range(KC):
                        nc.tensor.matmul(
                            pq[:, 0:BL],
                            lhsT=WpT[:, k, mc * P:(mc + 1) * P],
                            rhs=decTt[:, k, 0:BL],
                            start=(k == 0),
                            stop=(k == KC - 1),
                        )
                    nc.vector.tensor_copy(out=qprojT[:, mc, :], in_=pq[:, 0:BL])

                # all-ones row for partition-broadcast matmuls (K=1)
                ones1 = consts.tile([1, P], bf16, tag="ones1", name="ones1")
                nc.vector.memset(ones1[:], 1.0)
                # f32 identity for the final [p,k]->[k,p] PE transpose
                identf = consts.tile([P, P], fp32, tag="identf", name="identf")
                make_identity(nc, identf[:])
                # final output staging: outstage[k, b*128+f] = out[b, k*128+f]
                outstage = consts.tile([KC, BL * P], fp32, tag="outstage",
                                       name="outstage")
                if variant != "full":
                    nc.vector.memset(outstage[:], 0.0)

                # ---------- main loop ----------
                def finalize_a(den):
                    # den total -> bf16 scalar (DVE only; emitted at b end)
                    dtot = sm.tile([1, 1], fp32, tag="dtot", name="dtot")
                    nc.vector.reduce_sum(out=dtot[:], in_=den[:],
                                         axis=mybir.AxisListType.X)
                    dbf = sm.tile([1, 1], bf16, tag="dbf", name="dbf", bufs=2)
                    nc.vector.tensor_copy(out=dbf[:], in_=dtot[:])
                    return dbf

                def finalize_b(b, acc, dbf):
                    # out[b] = num / den.  The PE transpose depends only on
                    # acc (done at b end) and the den broadcast only on dbf,
                    # so neither stalls the PE stream when emitted later.
                    dps = psum_n.tile([P, ST], fp32, tag="pbc", name="dps")
                    nc.tensor.matmul(
                        dps[:, 0:1], lhsT=ones1[:], rhs=dbf[:],
                        start=True, stop=True
                    )
                    cps = psum_s.tile([KC, 512], fp32, tag="ps", name="cps")
                    nc.tensor.transpose(cps[:, 0:P], acc[:], identf[:])
                    inv = sm.tile([KC, 1], fp32, tag="inv", name="inv")
                    nc.vector.reciprocal(out=inv[:], in_=dps[0:KC, 0:1])
                    nc.vector.tensor_scalar_mul(
                        outstage[:, b * P:(b + 1) * P], cps[0:KC, 0:P], inv[:]
                    )

                # ---- main loop with tail batching ----
                # If the tail (s_pad % 512) divides 512, the tails of
                # g_n = 512//tail consecutive batch rows are packed into ONE
                # full 512-column tile: the projection/score matmuls are
                # batch-agnostic (same We, same v), only the tanh bias, the
                # mask and the softmax segmentation are per-b — applied on
                # column segments. This removes all narrow (N<512) matmuls.
                def proj_tile(xh, size, bias_segs, ps, matmul_only=False):
                    # proj^T + fused tanh(x + qproj_b) per column segment.
                    # The score matmul for group mc is emitted AFTER group
                    # mc+1's projection matmuls: the in-order PE queue would
                    # otherwise stall on the tanh(mc) -> score(mc)
                    # dependency eight times per tile.
                    def score(mc, th):
                        # scores[0, s'] += sum_h v[h] * tanh[h, s']
                        nc.tensor.matmul(
                            ps[:, 0:size],
                            lhsT=vT[:, mc, 0:1],
                            rhs=th[:, 0:size],
                            start=(mc == 0),
                            stop=(mc == KC - 1),
                        )

                    prev = None
                    for mc in range(KC):
                        pj = psum_pj.tile([P, 512], fp32, tag="pj", name="pj")
                        for k in range(KC):
                            nc.tensor.matmul(
                                pj[:, 0:size],
                                lhsT=WeT[:, k, mc * P:(mc + 1) * P],
                                rhs=xh[:, k, 0:size],
                                start=(k == 0),
                                stop=(k == KC - 1),
                            )
                        if matmul_only:
                            continue  # timing-only: raw matmul stream
                        if prev is not None:
                            score(*prev)
                        th = th_pool.tile([P, ST], bf16, tag="th", name="th")
                        for (b, so, sz) in bias_segs:
                            nc.scalar.activation(
                                out=th[:, so:so + sz],
                                in_=pj[:, so:so + sz],
                                func=Tanh,
                                bias=qprojT[:, mc, b:b + 1],
                                scale=1.0,
                            )
                        prev = (mc, th)
                    if not matmul_only:
                        score(*prev)

                def numer_tile(xh, size, pv, segs):
                    """Shared broadcast + products; per-(b,segment)
                    reduction into each b's acc. segs: (acc, s0, slen)."""
                    pbc = psum_n.tile([P, ST], fp32, tag="pbc", name="pbc")
                    nc.tensor.matmul(
                        pbc[:, 0:size], lhsT=ones1[:], rhs=pv[:, 0:size],
                        start=True, stop=True
                    )
                    tmp = pp_pool.tile([P, KC, ST], bf16, tag="ntmp",
                                       name="ntmp", bufs=2)
                    nc.vector.tensor_tensor(
                        out=tmp[:, :, 0:size],
                        in0=xh[:, :, 0:size],
                        in1=pbc[:, 0:size][:, None, :].to_broadcast(
                            [P, KC, size]),
                        op=mult,
                    )
                    for (acc, s0, slen) in segs:
                        red = sm.tile([P, KC], fp32, tag="red", name="red",
                                      bufs=2)
                        nc.vector.reduce_sum(
                            out=red[:], in_=tmp[:, :, s0:s0 + slen],
                            axis=mybir.AxisListType.X
                        )
                        nc.vector.tensor_add(out=acc[:], in0=acc[:],
                                             in1=red[:])

                ngroups = BL // g_n
                nden = nfull + (1 if tail else 0)
                pending = []
                for g in range(ngroups):
                    bs = list(range(g * g_n, (g + 1) * g_n))
                    accs, dens, mrows = {}, {}, {}
                    for b in bs:
                        accs[b] = sm.tile([P, KC], fp32, tag="acc",
                                          name="acc", bufs=2 * g_n)
                        nc.vector.memset(accs[b][:], 0.0)
                        dens[b] = sm.tile([1, max(nden, 1)], fp32, tag="den",
                                          name="den", bufs=2 * g_n)
                        # mask row via SWDGE: the sync/HWDGE queue stays in
                        # xbar transpose mode all rep long (copy<->transpose
                        # switches drain the xbar ~3us, not in the model)
                        mrows[b] = sm.tile([1, S], i32, tag="mrow",
                                           name="mrow", bufs=g_n + 1)
                        nc.gpsimd.dma_start(out=mrows[b][:, 0:s_pad],
                                            in_=msk[b:b + 1, :])
                    for b in bs:
                        for st in range(nper):
                            size = sizes[st]
                            if (b, st) in pre:
                                xh = pre.pop((b, st))
                            else:
                                xh = load_tile(b, st)

                            if pending:
                                finalize_b(*pending.pop(0))
                            if variant == "loads":
                                continue
                            ps = psum_s.tile([1, 512], fp32, tag="ps",
                                             name="ps")
                            proj_tile(xh, size, [(b, 0, size)], ps,
                                      matmul_only=(variant == "projonly"))
                            if variant in ("pescore", "projonly"):
                                continue

                            ex = sm.tile([1, ST], fp32, tag="ex", name="ex",
                                         bufs=2)
                            nc.scalar.activation(out=ex[:, 0:size],
                                                 in_=ps[:, 0:size], func=Exp)
                            mf = sm.tile([1, ST], fp32, tag="mf", name="mf",
                                         bufs=2)
                            nc.vector.tensor_copy(
                                out=mf[:, 0:size],
                                in_=mrows[b][0:1, offs[st]:offs[st] + size]
                            )
                            # p = ex * maskf (bf16); den[st] = sum_s p
                            pv = pp_pool.tile([1, ST], bf16, tag="pv",
                                              name="pv")
                            nc.vector.tensor_tensor(
                                out=pv[:, 0:size], in0=ex[:, 0:size],
                                in1=mf[:, 0:size], op=mult
                            )
                            nc.vector.reduce_sum(
                                out=dens[b][:, st:st + 1],
                                in_=pv[:, 0:size],
                                axis=mybir.AxisListType.X,
                            )
                            numer_tile(xh, size, pv, [(accs[b], 0, size)])

                    if g_n > 1:
                        # shared tail tile: g_n tails side by side, full 512
                        xh = xt_pool.tile([P, KC, ST], bf16, tag="xt",
                                          name="xt")
                        if LOAD_MODE == "swdge":
                            for j, b in enumerate(bs):
                                nc.gpsimd.dma_start(
                                    out=xh[:, :, j * tail:(j + 1) * tail],
                                    in_=enc[b, :,
                                            offs[nfull]:offs[nfull] + tail
                                            ].rearrange("(k p) t -> p k t",
                                                        p=P),
                                )
                        else:
                            xf = xf_pool.tile([P, KC, ST], fp32, tag="xf",
                                              name="xf")
                            for j, b in enumerate(bs):
                                nc.sync.dma_start(
                                    out=xf[:, :, j * tail:(j + 1) * tail],
                                    in_=enc[b, :,
                                            offs[nfull]:offs[nfull] + tail
                                            ].rearrange("(k p) t -> p k t",
                                                        p=P),
                                )
                            nc.gpsimd.tensor_copy(out=xh[:], in_=xf[:])
                        if pending:
                            finalize_b(*pending.pop(0))
                        if variant != "loads":
                            ps = psum_s.tile([1, 512], fp32, tag="ps",
                                             name="ps")
                            proj_tile(xh, gsz,
                                      [(b, j * tail, tail)
                                       for j, b in enumerate(bs)], ps,
                                      matmul_only=(variant == "projonly"))
                        if variant == "full":
                            ex = sm.tile([1, ST], fp32, tag="ex", name="ex",
                                         bufs=2)
                            nc.scalar.activation(out=ex[:, 0:gsz],
                                                 in_=ps[:, 0:gsz], func=Exp)
                            mf = sm.tile([1, ST], fp32, tag="mf", name="mf",
                                         bufs=2)
                            for j, b in enumerate(bs):
                                nc.vector.tensor_copy(
                                    out=mf[:, j * tail:(j + 1) * tail],
                                    in_=mrows[b][0:1,
                                                 offs[nfull]:offs[nfull]
                                                 + tail]
                                )
                            pv = pp_pool.tile([1, ST], bf16, tag="pv",
                                              name="pv")
                            nc.vector.tensor_tensor(
                                out=pv[:, 0:gsz], in0=ex[:, 0:gsz],
                                in1=mf[:, 0:gsz], op=mult
                            )
                            for j, b in enumerate(bs):
                                nc.vector.reduce_sum(
                                    out=dens[b][:, nfull:nfull + 1],
                                    in_=pv[:, j * tail:(j + 1) * tail],
                                    axis=mybir.AxisListType.X,
                                )
                            numer_tile(xh, gsz, pv,
                                       [(accs[b], j * tail, tail)
                                        for j, b in enumerate(bs)])

                    if variant == "full":
                        for b in bs:
                            pending.append((b, accs[b], finalize_a(dens[b])))
                while pending:
                    finalize_b(*pending.pop(0))

                # final store via SWDGE for the same xbar-mode reason
                nc.gpsimd.dma_start(
                    out=out[:, :].rearrange("b (k f) -> k b f", k=KC),
                    in_=outstage[:].rearrange("k (b f) -> k b f", b=BL),
                )

    _legalize_dma_waits(nc)
    return nc


def _legalize_dma_waits(nc):
    """This container's walrus enforces per-instruction sync budgets the Tile
    pipeline does not respect: most ISA encodings carry at most ONE sync-wait
    slot (EventSemaphore holds two), and the 64-byte-padded
    EVENT_SEMAPHORE_RANGE_CLEAR InstISA is rejected outright.  Legalize after
    Tile: move excess waits onto standalone EventSemaphore instructions
    inserted just before the instruction on the same engine stream (the
    sequencer executes them in order, so the instruction still issues only
    after all its waits are satisfied), and drop the teardown range-clear
    (this NEFF executes once; semaphores are not recycled afterwards)."""
    import concourse.mybir as mybir
    import bass_rust

    nev = [0]

    def mkev(engine, waits, updates=()):
        ev = mybir.InstEventSemaphore(name=f"evw-{nev[0]}", ins=[], outs=[])
        nev[0] += 1
        ev.engine = engine
        ev.sync_info = bass_rust.SyncInfo(
            on_wait=list(waits), on_update=list(updates)
        )
        return ev

    for blk in nc.m.functions[0].blocks:
        insts = blk.instructions
        new = []
        for inst in insts:
            t = type(inst).__name__
            si = getattr(inst, "sync_info", None)
            cap = 2 if t == "InstEventSemaphore" else 1
            if si is not None and len(si.on_wait) > cap:
                waits = list(si.on_wait)
                extra, keep = waits[:-cap], waits[-cap:]
                for j in range(0, len(extra), 2):
                    new.append(mkev(inst.engine, extra[j:j + 2]))
                inst.sync_info = bass_rust.SyncInfo(
                    on_wait=keep, on_update=list(si.on_update)
                )
            if t == "InstISA" and getattr(inst, "op_name", "") == (
                "EVENT_SEMAPHORE_RANGE_CLEAR"
            ):
                # Replace with per-semaphore EventSemaphore writes of 0: the
                # tail barrier recycles these sem ids and expects them
                # cleared; dropping the clear leaves DMA-lane counts behind
                # and lets the final barrier pass early (intermittent
                # exec-unit errors with the output store still in flight).
                ib = list(inst.instr)
                lo, hi = ib[13], ib[14]
                for s in range(lo, hi + 1):
                    new.append(mkev(inst.engine, [], [bass_rust.SyncUpdate(
                        sync_type="semaphore", id=s, ant_name=f"semclr{s}",
                        update_mode="sem-wr-imm", update_value=0,
                        update_reg=None)]))
                continue
            new.append(inst)
        try:
            blk.instructions = new
        except Exception:
            insts.clear()
            insts.extend(new)


def _get_nc(s_pad=S, reps=1):
    key = (s_pad, reps)
    if key not in _CACHE:
        _CACHE[key] = _build_bass(reps=reps, s_pad=s_pad)
    return _CACHE[key]


def _make_in_maps(inputs):
    """Shard over batch; host-side layout prep (transpose weights, compact
    the masked s axis — pure gather/permutation, no arithmetic).
    Returns (in_maps, s_pad)."""
    enc = np.ascontiguousarray(np.asarray(inputs["encoder_states"],
                                          dtype=np.float32))
    dec = np.ascontiguousarray(np.asarray(inputs["decoder_prev_state"],
                                          dtype=np.float32))
    msk = np.ascontiguousarray(np.asarray(inputs["mask"], dtype=np.int32))
    WpT = np.ascontiguousarray(np.asarray(inputs["Wp"], dtype=np.float32).T)
    WeT = np.ascontiguousarray(np.asarray(inputs["We"], dtype=np.float32).T)
    v = np.ascontiguousarray(np.asarray(inputs["v"], dtype=np.float32))

    cnts = (msk != 0).sum(axis=1)
    # pad only to the max row count: s' is a free (column) dimension in the
    # h-major device layout, so no alignment granularity is required
    s_pad = int(min(S, max(8, int(cnts.max()))))
    # enc_t[b, h, s'] = enc[idx_b[s'], b, h]: compacted along s AND
    # transposed to h-major so device tiles load with contiguous lines
    enc_t = np.zeros((B, H, s_pad), dtype=np.float32)
    msk_c = np.zeros((B, s_pad), dtype=np.int32)
    for b in range(B):
        idx = np.nonzero(msk[b])[0] if s_pad < S else np.arange(S)
        n = len(idx)
        enc_t[b, :, :n] = enc[idx, b, :].T
        msk_c[b, :n] = msk[b, idx]
    in_maps = []
    for i in range(NCORES):
        sl = slice(i * BL, (i + 1) * BL)
        in_maps.append(
            {
                "encoder_states": np.ascontiguousarray(enc_t[sl, :, :]),
                "decoder_prev_state": np.ascontiguousarray(dec[sl, :]),
                "mask": np.ascontiguousarray(msk_c[sl, :]),
                "WpT": WpT,
                "WeT": WeT,
                "v": v,
            }
        )
    return in_maps, s_pad


def kernel_profiled(trace=False, **inputs):
    """Run on 8 cores; returns (full_output, BassKernelResults)."""
    from concourse.bass_utils import run_bass_kernel_spmd

    in_maps, s_pad = _make_in_maps(inputs)
    nc = _get_nc(s_pad)
    res = run_bass_kernel_spmd(nc, in_maps, core_ids=list(range(NCORES)),
                               trace=trace)
    out = np.concatenate([r["out"] for r in res.results], axis=0)
    return out.astype(np.float32), res


def kernel(**inputs):
    out, _ = kernel_profiled(trace=False, **inputs)
    return out
